# revision 17
# baseline (speedup 1.0000x reference)
"""Trainium2 Bass kernel for AssignClsLabel (clipped-IoU >= 0.7 proposal labeling).

Problem: bboxess [8, 65536, 4] f32, gt_bboxess [8, 64, 4] f32,
gt_counts/counts [8,1] int. Output labels [8, 65536, 1] int (0/1).

Only proposals n < count_b and gts a < gt_count_b matter (~16% of the
full N*A grid here), so work is packed as UNITS = (batch b, chunk of
Q=704 proposals, group of G=4 gts) spread over 8 cores x 128
partitions x T iterations; every partition-slot carries its own
per-gt scalar columns (tensor_scalar / activation bias operands are
per-partition), so different partitions process different batches in
the same instruction.

Device math per pair (all f32; 0 label flips vs reference on the
fixed dataset):
    clip(v) into [g1,g2]; dy = clip(y2)-clip(y1); dx likewise
    i = dy*dx;  c = area + ga
    fire <=> (i - (12/17)c)^2 <= ((5/17)c)^2 <=> |i-(12/17)c|-(5/17)|c| <= 0
Engine split (rates measured on HW): DVE tensor_scalar 2-op ~0.6ns/elem
does clips for gts 0,1 (max,min fused); ACT (~1.0ns/elem) does relu-pair
clips for gts 2,3 plus the c-terms (Identity/Abs with fused input scale
and per-partition ga bias) and |i - c07|; DVE tensor_tensor (1.1ns/elem)
does dy/dx/i/ip/gg and the 4-gt min tree.
"""
import sys

import numpy as np

if "/opt/trn_rl_repo" not in sys.path:
    sys.path.insert(0, "/opt/trn_rl_repo")

import concourse.mybir as mybir
import concourse.tile as tile
from concourse import bacc
from concourse.bass_utils import run_bass_kernel_spmd

AOP = mybir.AluOpType
ACT = mybir.ActivationFunctionType
F32 = mybir.dt.float32

P = 128          # SBUF partitions
Q = 704          # proposals per work unit
G = 4            # gts per work unit
# clip routing: (gt, axis) pairs handled by DVE tensor_scalar; rest on ACT
TS_AXES = {(0, 0), (0, 1), (1, 0)}
N_CORES = 8
F1712 = float(np.float32(17.0 / 12.0))
F512 = float(np.float32(5.0 / 12.0))

# scal columns per slot, per gt j in 0..G-1
S_GY1 = 0 * G
S_GY2 = 1 * G
S_GX1 = 2 * G
S_GX2 = 3 * G
S_NGY1 = 4 * G   # -gy1
S_D21Y = 5 * G   # gy2 - gy1
S_NGX1 = 6 * G   # -gx1
S_E21X = 7 * G   # gx2 - gx1
S_GA = 8 * G     # ga
S_GA512 = 9 * G  # (5/12) ga
SCAL_W = 10 * G

FQ = 5 * Q       # feature width per slot: y1,y2,x1,x2,area


def make_plan(inputs):
    counts = inputs["counts"]
    gt_counts = inputs["gt_counts"]
    B = counts.shape[0]
    units = []   # (b, n0, L, gt_idx tuple)
    for b in range(B):
        cnt = int(counts[b, 0])
        gcnt = int(gt_counts[b, 0])
        if cnt <= 0 or gcnt <= 0:
            continue
        nchunks = -(-cnt // Q) if cnt >= Q else 1
        ngroups = -(-gcnt // G)
        for k in range(nchunks):
            n0 = min(k * Q, max(0, cnt - Q))
            L = min(Q, cnt - n0)
            for g in range(ngroups):
                a0 = min(g * G, max(0, gcnt - G))
                gt_idx = tuple(min(a0 + j, gcnt - 1) for j in range(G))
                units.append((b, n0, L, gt_idx))
    T = -(-len(units) // (N_CORES * P))
    return {"units": units, "T": T}


def build_graph(plan):
    T = plan["T"]
    nc = bacc.Bacc()
    feat_d = nc.declare_dram_parameter("feat", [P, T * FQ], F32, isOutput=False)
    scal_d = nc.declare_dram_parameter("scal", [P, T * SCAL_W], F32,
                                       isOutput=False)
    out_d = nc.declare_dram_parameter("out", [P, T * Q], F32, isOutput=True)

    with tile.TileContext(nc) as tc:
        with (
            tc.tile_pool(name="ft", bufs=2) as fp,
            tc.tile_pool(name="gt", bufs=2) as gp,
            tc.tile_pool(name="itp", bufs=5) as itp,
            tc.tile_pool(name="ipp", bufs=5) as ipp,
            tc.tile_pool(name="a1p", bufs=8) as a1p,
            tc.tile_pool(name="cpool", bufs=2) as cp,
            tc.tile_pool(name="ggp", bufs=5) as ggp,
            tc.tile_pool(name="sm", bufs=1) as sp,
        ):
            st = [dict() for _ in range(T)]

            def front(t):
                """DMAs + clips + cab for slot t."""
                d = st[t]
                ftile = fp.tile([P, FQ], F32, tag="feat", name=f"feat{t}")
                stile = fp.tile([P, SCAL_W], F32, tag="scal", name=f"scal{t}")
                nc.sync.dma_start(stile[:], scal_d[:, t * SCAL_W:
                                                   (t + 1) * SCAL_W])
                nc.sync.dma_start(ftile[:, 0:2 * Q],
                                  feat_d[:, t * FQ:t * FQ + 2 * Q])
                nc.sync.dma_start(ftile[:, 2 * Q:4 * Q],
                                  feat_d[:, t * FQ + 2 * Q:t * FQ + 4 * Q])
                nc.sync.dma_start(ftile[:, 4 * Q:5 * Q],
                                  feat_d[:, t * FQ + 4 * Q:(t + 1) * FQ])
                d["fy12"] = ftile[:, 0:2 * Q]
                d["fx12"] = ftile[:, 2 * Q:4 * Q]
                d["farea"] = ftile[:, 4 * Q:5 * Q]
                d["stile"] = stile

                def col(base, j, stile=stile):
                    return stile[:, base + j:base + j + 1]
                d["col"] = col

                cl = {}
                for j in range(G):
                    for ax in (0, 1):
                        fin = d["fy12"] if ax == 0 else d["fx12"]
                        tag = ("cly", "clx")[ax]
                        if (j, ax) in TS_AXES:
                            cc = gp.tile([P, 2 * Q], F32, tag=tag, bufs=2,
                                         name=f"ts{t}_{j}_{ax}")
                            lo = col((S_GY1, S_GX1)[ax], j)
                            hi = col((S_GY2, S_GX2)[ax], j)
                            nc.vector.tensor_scalar(cc[:], fin, lo, hi,
                                                    AOP.max, AOP.min)
                            cl[(j, ax)] = (cc, True)
                        else:
                            r1 = gp.tile([P, 2 * Q], F32, tag="r1", bufs=2,
                                         name=f"r1_{t}_{j}_{ax}")
                            cc = gp.tile([P, 2 * Q], F32, tag=tag, bufs=2,
                                         name=f"r2_{t}_{j}_{ax}")
                            b1 = col((S_NGY1, S_NGX1)[ax], j)
                            b2 = col((S_D21Y, S_E21X)[ax], j)
                            nc.scalar.activation(r1[:], fin, ACT.Relu, bias=b1)
                            nc.scalar.activation(cc[:], r1[:], ACT.Relu,
                                                 bias=b2, scale=-1.0)
                            cl[(j, ax)] = (cc, False)
                d["cl"] = cl
                cab = cp.tile([P, G * Q], F32, tag="cab", name=f"cab{t}")
                for j in range(G):
                    nc.scalar.activation(cab[:, j * Q:(j + 1) * Q], d["farea"],
                                         ACT.Abs, bias=col(S_GA512, j),
                                         scale=F512)
                d["cab"] = cab

            def dys(t):
                d = st[t]
                out = []
                for j in range(G):
                    dy = gp.tile([P, Q], F32, tag="dy", bufs=5,
                                 name=f"dy{t}_{j}")
                    dx = gp.tile([P, Q], F32, tag="dx", bufs=5,
                                 name=f"dx{t}_{j}")
                    for ax, dd in ((0, dy), (1, dx)):
                        cc, direct = d["cl"][(j, ax)]
                        if direct:   # dy = clip(v2) - clip(v1)
                            nc.vector.tensor_tensor(dd[:], cc[:, Q:2 * Q],
                                                    cc[:, 0:Q], AOP.subtract)
                        else:        # dy = r2(v1) - r2(v2)
                            nc.vector.tensor_tensor(dd[:], cc[:, 0:Q],
                                                    cc[:, Q:2 * Q],
                                                    AOP.subtract)
                    out.append((dy, dx))
                d["dys"] = out

            def its_ips(t):
                d = st[t]
                its = []
                for j in range(G):
                    dy, dx = d["dys"][j]
                    it = itp.tile([P, Q], F32, tag="it", name=f"it{t}_{j}")
                    # i' = (17/12) * dy * dx
                    nc.vector.scalar_tensor_tensor(
                        it[:], dy[:], F1712, dx[:], AOP.mult, AOP.mult)
                    its.append(it)
                ips = []
                for j in range(G):
                    ip = ipp.tile([P, Q], F32, tag="ip", name=f"ip{t}_{j}")
                    # ip = (i' - ga) - area
                    nc.vector.scalar_tensor_tensor(
                        ip[:], its[j][:], d["col"](S_GA, j), d["farea"],
                        AOP.subtract, AOP.subtract)
                    ips.append(ip)
                d["ips"] = ips

            def a1s(t):
                d = st[t]
                out = []
                for j in range(G):
                    a1 = a1p.tile([P, Q], F32, tag="a1", name=f"a1{t}_{j}")
                    nc.scalar.activation(a1[:], d["ips"][j][:], ACT.Abs)
                    out.append(a1)
                d["a1s"] = out

            def tail(t):
                d = st[t]
                ggs = []
                for j in range(G):
                    gg = ggp.tile([P, Q], F32, tag="gg", name=f"gg{t}_{j}")
                    nc.vector.tensor_tensor(gg[:], d["a1s"][j][:],
                                            d["cab"][:, j * Q:(j + 1) * Q],
                                            AOP.subtract)
                    ggs.append(gg)
                t1 = sp.tile([P, Q], F32, tag="t1", name=f"t1{t}")
                t2 = sp.tile([P, Q], F32, tag="t2", name=f"t2{t}")
                macc = sp.tile([P, Q], F32, tag="macc", name=f"macc{t}")
                nc.vector.tensor_tensor(t1[:], ggs[0][:], ggs[1][:], AOP.min)
                nc.vector.tensor_tensor(t2[:], ggs[2][:], ggs[3][:], AOP.min)
                nc.vector.tensor_tensor(macc[:], t1[:], t2[:], AOP.min)
                nc.sync.dma_start(out_d[:, t * Q:(t + 1) * Q], macc[:])

            # software pipeline across slots: slot t+1's front work fills
            # the gaps left by slot t's cross-engine round trips.
            front(0)
            for t in range(T):
                dys(t)
                if t + 1 < T:
                    front(t + 1)
                its_ips(t)
                a1s(t)
                if t > 0:
                    tail(t - 1)
            tail(T - 1)

    nc.finalize()
    return nc


def host_prep(inputs, plan):
    bboxess = np.asarray(inputs["bboxess"], dtype=np.float32)
    gt_bboxess = np.asarray(inputs["gt_bboxess"], dtype=np.float32)
    units = plan["units"]
    T = plan["T"]

    y1 = bboxess[:, :, 0]
    x1 = bboxess[:, :, 1]
    y2 = bboxess[:, :, 2]
    x2 = bboxess[:, :, 3]
    area = ((y2 - y1) * (x2 - x1)).astype(np.float32)
    gy1 = gt_bboxess[:, :, 0]
    gx1 = gt_bboxess[:, :, 1]
    gy2 = gt_bboxess[:, :, 2]
    gx2 = gt_bboxess[:, :, 3]
    ga = ((gy2 - gy1) * (gx2 - gx1)).astype(np.float32)
    gtab = {
        S_GY1: gy1, S_GY2: gy2, S_GX1: gx1, S_GX2: gx2,
        S_NGY1: -gy1, S_D21Y: (gy2 - gy1), S_NGX1: -gx1, S_E21X: (gx2 - gx1),
        S_GA: ga,
        S_GA512: (np.float32(F512) * ga).astype(np.float32),
    }
    feats = (y1, y2, x1, x2, area)

    in_maps = []
    for c in range(N_CORES):
        feat = np.zeros((P, T * FQ), dtype=np.float32)
        scal = np.zeros((P, T * SCAL_W), dtype=np.float32)
        for t in range(T):
            for p in range(P):
                u = t * (N_CORES * P) + p * N_CORES + c
                if u >= len(units):
                    u = 0
                b, n0, L, gt_idx = units[u]
                base = t * FQ
                for fi, f in enumerate(feats):
                    dst = feat[p, base + fi * Q: base + fi * Q + L]
                    dst[:] = f[b, n0:n0 + L]
                    if L < Q:
                        feat[p, base + fi * Q + L: base + (fi + 1) * Q] = \
                            f[b, n0]
                sb = t * SCAL_W
                for fld, tab in gtab.items():
                    for j in range(G):
                        scal[p, sb + fld + j] = tab[b, gt_idx[j]]
        in_maps.append({"feat": feat, "scal": scal})
    return in_maps


def host_post(results, plan, inputs):
    counts = inputs["counts"]
    out_dtype = np.int64 if counts.dtype == np.int64 else np.int32
    B = counts.shape[0]
    N = inputs["bboxess"].shape[1]
    units = plan["units"]
    T = plan["T"]
    labels = np.zeros((B, N, 1), dtype=out_dtype)
    for c in range(N_CORES):
        o = results[c]["out"]   # [P, T*Q] f32 min-margin values
        fire = o <= 0.0
        for t in range(T):
            for p in range(P):
                u = t * (N_CORES * P) + p * N_CORES + c
                if u >= len(units):
                    continue
                b, n0, L, _ = units[u]
                seg = fire[p, t * Q: t * Q + L]
                np.logical_or(labels[b, n0:n0 + L, 0], seg,
                              out=labels[b, n0:n0 + L, 0],
                              casting="unsafe")
    return labels


def _axon_reset():
    import ctypes
    try:
        lib = ctypes.CDLL("/opt/axon/libaxon_pjrt.so")
        lib.axon_reset.restype = ctypes.c_int64
        lib.axon_reset()
    except Exception:
        pass


def kernel(bboxess, gt_bboxess, gt_counts, counts):
    inputs = {"bboxess": np.asarray(bboxess),
              "gt_bboxess": np.asarray(gt_bboxess),
              "gt_counts": np.asarray(gt_counts),
              "counts": np.asarray(counts)}
    plan = make_plan(inputs)
    nc = build_graph(plan)
    in_maps = host_prep(inputs, plan)
    try:
        res = run_bass_kernel_spmd(nc, in_maps, core_ids=list(range(N_CORES)))
    except Exception:
        _axon_reset()
        res = run_bass_kernel_spmd(nc, in_maps, core_ids=list(range(N_CORES)))
    return host_post(res.results, plan, inputs)


# revision 18
# speedup vs baseline: 1.0806x; 1.0806x over previous
"""Trainium2 Bass kernel for AssignClsLabel (clipped-IoU >= 0.7 proposal labeling).

Problem: bboxess [8, 65536, 4] f32, gt_bboxess [8, 64, 4] f32,
gt_counts/counts [8,1] int. Output labels [8, 65536, 1] int (0/1).

Only proposals n < count_b and gts a < gt_count_b matter (~16% of the
full N*A grid here), so work is packed as UNITS = (batch b, chunk of
Q=704 proposals, group of G=4 gts) spread over 8 cores x 128
partitions x T iterations; every partition-slot carries its own
per-gt scalar columns (tensor_scalar / activation bias operands are
per-partition), so different partitions process different batches in
the same instruction.

Device math per pair (all f32; 0 label flips vs reference on the
fixed dataset):
    clip(v) into [g1,g2]; dy = clip(y2)-clip(y1); dx likewise
    i = dy*dx;  c = area + ga
    fire <=> (i - (12/17)c)^2 <= ((5/17)c)^2 <=> |i-(12/17)c|-(5/17)|c| <= 0
Engine split (rates measured on HW): DVE tensor_scalar 2-op ~0.6ns/elem
does clips for gts 0,1 (max,min fused); ACT (~1.0ns/elem) does relu-pair
clips for gts 2,3 plus the c-terms (Identity/Abs with fused input scale
and per-partition ga bias) and |i - c07|; DVE tensor_tensor (1.1ns/elem)
does dy/dx/i/ip/gg and the 4-gt min tree.
"""
import sys

import numpy as np

if "/opt/trn_rl_repo" not in sys.path:
    sys.path.insert(0, "/opt/trn_rl_repo")

import concourse.mybir as mybir
import concourse.tile as tile
from concourse import bacc
from concourse.bass_utils import run_bass_kernel_spmd

AOP = mybir.AluOpType
ACT = mybir.ActivationFunctionType
F32 = mybir.dt.float32

P = 128          # SBUF partitions
Q = 704          # proposals per work unit
G = 4            # gts per work unit
# clip routing: (gt, axis) pairs handled by DVE tensor_scalar; rest on ACT
TS_AXES = {(0, 0), (0, 1), (1, 0)}
N_CORES = 8
F1712 = float(np.float32(17.0 / 12.0))
F512 = float(np.float32(5.0 / 12.0))

# scal columns per slot, per gt j in 0..G-1
S_GY1 = 0 * G
S_GY2 = 1 * G
S_GX1 = 2 * G
S_GX2 = 3 * G
S_NGY1 = 4 * G   # -gy1
S_D21Y = 5 * G   # gy2 - gy1
S_NGX1 = 6 * G   # -gx1
S_E21X = 7 * G   # gx2 - gx1
S_GA = 8 * G     # ga
S_GA512 = 9 * G  # (5/12) ga
SCAL_W = 10 * G

FQ = 5 * Q       # feature width per slot: y1,y2,x1,x2,area


def make_plan(inputs):
    counts = inputs["counts"]
    gt_counts = inputs["gt_counts"]
    B = counts.shape[0]
    units = []   # (b, n0, L, gt_idx tuple)
    for b in range(B):
        cnt = int(counts[b, 0])
        gcnt = int(gt_counts[b, 0])
        if cnt <= 0 or gcnt <= 0:
            continue
        nchunks = -(-cnt // Q) if cnt >= Q else 1
        ngroups = -(-gcnt // G)
        for k in range(nchunks):
            n0 = min(k * Q, max(0, cnt - Q))
            L = min(Q, cnt - n0)
            for g in range(ngroups):
                a0 = min(g * G, max(0, gcnt - G))
                gt_idx = tuple(min(a0 + j, gcnt - 1) for j in range(G))
                units.append((b, n0, L, gt_idx))
    T = -(-len(units) // (N_CORES * P))
    return {"units": units, "T": T}


def build_graph(plan):
    T = plan["T"]
    nc = bacc.Bacc()
    feat_d = nc.declare_dram_parameter("feat", [P, T * FQ], F32, isOutput=False)
    scal_d = nc.declare_dram_parameter("scal", [P, T * SCAL_W], F32,
                                       isOutput=False)
    out_d = nc.declare_dram_parameter("out", [P, T * Q], F32, isOutput=True)

    with tile.TileContext(nc) as tc:
        with (
            tc.tile_pool(name="ft", bufs=2) as fp,
            tc.tile_pool(name="gt", bufs=2) as gp,
            tc.tile_pool(name="itp", bufs=5) as itp,
            tc.tile_pool(name="ipp", bufs=5) as ipp,
            tc.tile_pool(name="a1p", bufs=8) as a1p,
            tc.tile_pool(name="cpool", bufs=2) as cp,
            tc.tile_pool(name="ggp", bufs=5) as ggp,
            tc.tile_pool(name="sm", bufs=1) as sp,
        ):
            st = [dict() for _ in range(T)]

            def front(t):
                """DMAs + clips + cab for slot t."""
                d = st[t]
                ftile = fp.tile([P, FQ], F32, tag="feat", name=f"feat{t}")
                stile = fp.tile([P, SCAL_W], F32, tag="scal", name=f"scal{t}")
                nc.sync.dma_start(stile[:], scal_d[:, t * SCAL_W:
                                                   (t + 1) * SCAL_W])
                nc.sync.dma_start(ftile[:, 0:2 * Q],
                                  feat_d[:, t * FQ:t * FQ + 2 * Q])
                nc.sync.dma_start(ftile[:, 2 * Q:4 * Q],
                                  feat_d[:, t * FQ + 2 * Q:t * FQ + 4 * Q])
                nc.sync.dma_start(ftile[:, 4 * Q:5 * Q],
                                  feat_d[:, t * FQ + 4 * Q:(t + 1) * FQ])
                d["fy12"] = ftile[:, 0:2 * Q]
                d["fx12"] = ftile[:, 2 * Q:4 * Q]
                d["farea"] = ftile[:, 4 * Q:5 * Q]
                d["stile"] = stile

                def col(base, j, stile=stile):
                    return stile[:, base + j:base + j + 1]
                d["col"] = col

                cl = {}
                for j in range(G):
                    for ax in (0, 1):
                        fin = d["fy12"] if ax == 0 else d["fx12"]
                        tag = ("cly", "clx")[ax]
                        if (j, ax) in TS_AXES:
                            cc = gp.tile([P, 2 * Q], F32, tag=tag, bufs=2,
                                         name=f"ts{t}_{j}_{ax}")
                            lo = col((S_GY1, S_GX1)[ax], j)
                            hi = col((S_GY2, S_GX2)[ax], j)
                            nc.vector.tensor_scalar(cc[:], fin, lo, hi,
                                                    AOP.max, AOP.min)
                            cl[(j, ax)] = (cc, True)
                        else:
                            r1 = gp.tile([P, 2 * Q], F32, tag="r1", bufs=2,
                                         name=f"r1_{t}_{j}_{ax}")
                            cc = gp.tile([P, 2 * Q], F32, tag=tag, bufs=2,
                                         name=f"r2_{t}_{j}_{ax}")
                            b1 = col((S_NGY1, S_NGX1)[ax], j)
                            b2 = col((S_D21Y, S_E21X)[ax], j)
                            nc.scalar.activation(r1[:], fin, ACT.Relu, bias=b1)
                            nc.scalar.activation(cc[:], r1[:], ACT.Relu,
                                                 bias=b2, scale=-1.0)
                            cl[(j, ax)] = (cc, False)
                d["cl"] = cl
                cab = cp.tile([P, G * Q], F32, tag="cab", name=f"cab{t}")
                for j in range(G):
                    nc.scalar.activation(cab[:, j * Q:(j + 1) * Q], d["farea"],
                                         ACT.Abs, bias=col(S_GA512, j),
                                         scale=F512)
                d["cab"] = cab

            def dys(t):
                d = st[t]
                out = []
                for j in range(G):
                    dy = gp.tile([P, Q], F32, tag="dy", bufs=4,
                                 name=f"dy{t}_{j}")
                    dx = gp.tile([P, Q], F32, tag="dx", bufs=4,
                                 name=f"dx{t}_{j}")
                    for ax, dd in ((0, dy), (1, dx)):
                        cc, direct = d["cl"][(j, ax)]
                        if direct:   # dy = clip(v2) - clip(v1)
                            nc.vector.tensor_tensor(dd[:], cc[:, Q:2 * Q],
                                                    cc[:, 0:Q], AOP.subtract)
                        else:        # dy = r2(v1) - r2(v2)
                            nc.vector.tensor_tensor(dd[:], cc[:, 0:Q],
                                                    cc[:, Q:2 * Q],
                                                    AOP.subtract)
                    out.append((dy, dx))
                d["dys"] = out

            def its_ips(t):
                d = st[t]
                its = []
                for j in range(G):
                    dy, dx = d["dys"][j]
                    it = itp.tile([P, Q], F32, tag="it", name=f"it{t}_{j}")
                    # i' = (17/12) * dy * dx
                    nc.vector.scalar_tensor_tensor(
                        it[:], dy[:], F1712, dx[:], AOP.mult, AOP.mult)
                    its.append(it)
                ips = []
                for j in range(G):
                    ip = ipp.tile([P, Q], F32, tag="ip", name=f"ip{t}_{j}")
                    # ip = (i' - ga) - area
                    nc.vector.scalar_tensor_tensor(
                        ip[:], its[j][:], d["col"](S_GA, j), d["farea"],
                        AOP.subtract, AOP.subtract)
                    ips.append(ip)
                d["ips"] = ips

            def a1s(t):
                d = st[t]
                out = []
                for j in range(G):
                    a1 = a1p.tile([P, Q], F32, tag="a1", name=f"a1{t}_{j}")
                    nc.scalar.activation(a1[:], d["ips"][j][:], ACT.Abs)
                    out.append(a1)
                d["a1s"] = out

            def tail(t):
                d = st[t]
                ggs = []
                for j in range(G):
                    gg = ggp.tile([P, Q], F32, tag="gg", name=f"gg{t}_{j}")
                    nc.vector.tensor_tensor(gg[:], d["a1s"][j][:],
                                            d["cab"][:, j * Q:(j + 1) * Q],
                                            AOP.subtract)
                    ggs.append(gg)
                t1 = sp.tile([P, Q], F32, tag="t1", name=f"t1{t}")
                t2 = sp.tile([P, Q], F32, tag="t2", name=f"t2{t}")
                macc = sp.tile([P, Q], F32, tag="macc", name=f"macc{t}")
                nc.vector.tensor_tensor(t1[:], ggs[0][:], ggs[1][:], AOP.min)
                nc.vector.tensor_tensor(t2[:], ggs[2][:], ggs[3][:], AOP.min)
                nc.vector.tensor_tensor(macc[:], t1[:], t2[:], AOP.min)
                nc.sync.dma_start(out_d[:, t * Q:(t + 1) * Q], macc[:])

            for t in range(T):
                front(t)
                dys(t)
                its_ips(t)
                a1s(t)
                tail(t)

    nc.finalize()
    return nc


def host_prep(inputs, plan):
    bboxess = np.asarray(inputs["bboxess"], dtype=np.float32)
    gt_bboxess = np.asarray(inputs["gt_bboxess"], dtype=np.float32)
    units = plan["units"]
    T = plan["T"]

    y1 = bboxess[:, :, 0]
    x1 = bboxess[:, :, 1]
    y2 = bboxess[:, :, 2]
    x2 = bboxess[:, :, 3]
    area = ((y2 - y1) * (x2 - x1)).astype(np.float32)
    gy1 = gt_bboxess[:, :, 0]
    gx1 = gt_bboxess[:, :, 1]
    gy2 = gt_bboxess[:, :, 2]
    gx2 = gt_bboxess[:, :, 3]
    ga = ((gy2 - gy1) * (gx2 - gx1)).astype(np.float32)
    gtab = {
        S_GY1: gy1, S_GY2: gy2, S_GX1: gx1, S_GX2: gx2,
        S_NGY1: -gy1, S_D21Y: (gy2 - gy1), S_NGX1: -gx1, S_E21X: (gx2 - gx1),
        S_GA: ga,
        S_GA512: (np.float32(F512) * ga).astype(np.float32),
    }
    feats = (y1, y2, x1, x2, area)

    in_maps = []
    for c in range(N_CORES):
        feat = np.zeros((P, T * FQ), dtype=np.float32)
        scal = np.zeros((P, T * SCAL_W), dtype=np.float32)
        for t in range(T):
            for p in range(P):
                u = t * (N_CORES * P) + p * N_CORES + c
                if u >= len(units):
                    u = 0
                b, n0, L, gt_idx = units[u]
                base = t * FQ
                for fi, f in enumerate(feats):
                    dst = feat[p, base + fi * Q: base + fi * Q + L]
                    dst[:] = f[b, n0:n0 + L]
                    if L < Q:
                        feat[p, base + fi * Q + L: base + (fi + 1) * Q] = \
                            f[b, n0]
                sb = t * SCAL_W
                for fld, tab in gtab.items():
                    for j in range(G):
                        scal[p, sb + fld + j] = tab[b, gt_idx[j]]
        in_maps.append({"feat": feat, "scal": scal})
    return in_maps


def host_post(results, plan, inputs):
    counts = inputs["counts"]
    out_dtype = np.int64 if counts.dtype == np.int64 else np.int32
    B = counts.shape[0]
    N = inputs["bboxess"].shape[1]
    units = plan["units"]
    T = plan["T"]
    labels = np.zeros((B, N, 1), dtype=out_dtype)
    for c in range(N_CORES):
        o = results[c]["out"]   # [P, T*Q] f32 min-margin values
        fire = o <= 0.0
        for t in range(T):
            for p in range(P):
                u = t * (N_CORES * P) + p * N_CORES + c
                if u >= len(units):
                    continue
                b, n0, L, _ = units[u]
                seg = fire[p, t * Q: t * Q + L]
                np.logical_or(labels[b, n0:n0 + L, 0], seg,
                              out=labels[b, n0:n0 + L, 0],
                              casting="unsafe")
    return labels


def _axon_reset():
    import ctypes
    try:
        lib = ctypes.CDLL("/opt/axon/libaxon_pjrt.so")
        lib.axon_reset.restype = ctypes.c_int64
        lib.axon_reset()
    except Exception:
        pass


def kernel(bboxess, gt_bboxess, gt_counts, counts):
    inputs = {"bboxess": np.asarray(bboxess),
              "gt_bboxess": np.asarray(gt_bboxess),
              "gt_counts": np.asarray(gt_counts),
              "counts": np.asarray(counts)}
    plan = make_plan(inputs)
    nc = build_graph(plan)
    in_maps = host_prep(inputs, plan)
    try:
        res = run_bass_kernel_spmd(nc, in_maps, core_ids=list(range(N_CORES)))
    except Exception:
        _axon_reset()
        res = run_bass_kernel_spmd(nc, in_maps, core_ids=list(range(N_CORES)))
    return host_post(res.results, plan, inputs)
